# revision 13
# baseline (speedup 1.0000x reference)
"""Trainium2 Bass kernel for a 6-layer encoder stack (nn_EncoderStack).

Strategy (8 NeuronCores, SPMD single program, per-core input shards):
  - Attention is tensor-parallel over heads (2 heads/core).  Everything that
    is per-token (residual adds, LayerNorms, FFN) is sequence-parallel
    (256 rows/core) with the FFN weights replicated.
  - Per layer the only collectives are one AllToAll (1 MB/rank, redistributes
    attention output from head-sharded to sequence-sharded) and one AllGather
    (1 MB/rank in, rebuilds the replicated transposed residual stream hT).
  - Scores are computed transposed (S^T = K Q^T, [keys, queries]) so the
    reference's log_softmax over axis=1 (queries) becomes a free-axis
    reduction.  log_softmax is applied lazily through the rank-1 identity
        attnT = V^T S^T - (V^T c) 1^T,   c[m] = logsumexp_n S^T[m, n]
    with the subtraction folded into the PSUM->SBUF copy as a per-partition
    tensor_scalar op.
  - All matmuls run as float32r (FP32 data, FP22 multiply, FP32 accumulate,
    full PE rate at free-dim >= 256).
  - LayerNorm = bn_stats/bn_aggr + rstd = Exp(-0.5 * Ln(var)) which stays in
    the ACT "natural_log_exp" table set (no table switches in steady state).
"""

import math
import sys
import os

import numpy as np

for _p in ("/opt/trn_rl_repo",):
    if _p not in sys.path:
        sys.path.insert(0, _p)

from concourse import bass, mybir, tile, bacc  # noqa: E402
from concourse import bass2jax  # noqa: E402

F32 = mybir.dt.float32
F32R = mybir.dt.float32r
AF = mybir.ActivationFunctionType
OP = mybir.AluOpType

L, H, N, DM, DK, DV, DFF, VOCAB = 6, 16, 2048, 1024, 64, 64, 4096, 32000
C = 8            # cores
HC = H // C      # heads per core
NS = N // C      # sequence shard per core
P = 128
RG = [list(range(C))]  # replica group: all 8 cores


# ---------------------------------------------------------------------------
# device program
# ---------------------------------------------------------------------------

def _build_program(has_bo_b2: bool, has_gb: bool):
    nc = bacc.Bacc(None, target_bir_lowering=False, num_devices=C)

    # ---- I/O ----
    h0_d = nc.declare_dram_parameter("h0", [NS, DM], F32, isOutput=False)
    pos_d = nc.declare_dram_parameter("pos", [NS, DM], F32, isOutput=False)
    wq_d = nc.declare_dram_parameter("wq", [L, DM, P], F32R, isOutput=False)
    wk_d = nc.declare_dram_parameter("wk", [L, DM, P], F32R, isOutput=False)
    wv_d = nc.declare_dram_parameter("wv", [L, DM, P], F32R, isOutput=False)
    bq_d = nc.declare_dram_parameter("bq", [L, P, 1], F32, isOutput=False)
    bk_d = nc.declare_dram_parameter("bk", [L, P, 1], F32, isOutput=False)
    bv_d = nc.declare_dram_parameter("bv", [L, P, 1], F32, isOutput=False)
    wo_d = nc.declare_dram_parameter("wo", [L, H * DV, DM], F32R, isOutput=False)
    w1_d = nc.declare_dram_parameter("w1", [L, DM, DFF], F32R, isOutput=False)
    b1_d = nc.declare_dram_parameter("b1", [L, DFF // P, P, 1], F32, isOutput=False)
    w2_d = nc.declare_dram_parameter("w2", [L, DFF, DM], F32R, isOutput=False)
    if has_bo_b2:
        bo_d = nc.declare_dram_parameter("bo_b", [L, P, DM], F32, isOutput=False)
        b2_d = nc.declare_dram_parameter("b2_b", [L, P, DM], F32, isOutput=False)
    if has_gb:
        g1_d = nc.declare_dram_parameter("g1s", [L, NS, DM], F32, isOutput=False)
        be1_d = nc.declare_dram_parameter("be1s", [L, NS, DM], F32, isOutput=False)
        g2_d = nc.declare_dram_parameter("g2s", [L, NS, DM], F32, isOutput=False)
        be2_d = nc.declare_dram_parameter("be2s", [L, NS, DM], F32, isOutput=False)
    out_d = nc.declare_dram_parameter("out", [NS, DM], F32, isOutput=True)

    # ---- internal DRAM (collective bounce buffers, per layer) ----
    cc_hT_in = [nc.dram_tensor(f"cc_hT_in{i}", [C * P, NS], F32) for i in range(L)]
    hT_all = [
        nc.dram_tensor(f"hT_all{i}", [C * C * P, NS], F32, addr_space="Shared")
        for i in range(L)
    ]
    cc_at_in = [nc.dram_tensor(f"cc_at_in{i}", [C * P, NS], F32) for i in range(L)]
    cc_at_out = [
        nc.dram_tensor(f"cc_at_out{i}", [C * P, NS], F32)
        for i in range(L)
    ]

    from concourse.masks import make_identity

    ES = bass.mybir.EngineType  # noqa: F841

    with tile.TileContext(nc) as tc:
        with (
            tc.tile_pool(name="const", bufs=1) as constp,
            tc.tile_pool(name="glob", bufs=1) as glob,
            tc.tile_pool(name="wq_g", bufs=1) as wq_g,
            tc.tile_pool(name="wo_g", bufs=1) as wo_g,
            tc.tile_pool(name="w12_g", bufs=3) as w12_g,
        ):
            idt = constp.tile([P, P], F32, tag="idt")
            make_identity(nc, idt[:])

            hbuf = [glob.tile([P, DM], F32, tag=f"hbuf{i}", name=f"hbuf{i}") for i in range(2)]

            # ---------------- stage 0: h0 + pos, transpose, AllGather -------
            with (
                tc.tile_pool(name="s0", bufs=2) as s0p,
                tc.tile_pool(name="s0ps", bufs=2, space="PSUM") as s0ps,
            ):
                for i in range(2):
                    t0 = s0p.tile([P, DM], F32, tag="h0t")
                    nc.sync.dma_start(t0[:], h0_d[i * P:(i + 1) * P, :])
                    t1 = s0p.tile([P, DM], F32, tag="post")
                    nc.sync.dma_start(t1[:], pos_d[i * P:(i + 1) * P, :])
                    nc.vector.tensor_add(hbuf[i][:], t0[:], t1[:])
                hTsh = s0p.tile([P, C, NS], F32R, tag="hTsh0")
                for i in range(2):
                    for dc in range(C):
                        tp = s0ps.tile([P, P], F32, tag="trps")
                        nc.tensor.transpose(
                            tp[:], hbuf[i][:, dc * P:(dc + 1) * P], idt[:]
                        )
                        nc.scalar.activation(
                            hTsh[:, dc, i * P:(i + 1) * P], tp[:], AF.Copy
                        )
                nc.sync.dma_start(
                    cc_hT_in[0].rearrange("(dc p) n -> p dc n", p=P).bitcast(F32R),
                    hTsh[:],
                )
                nc.gpsimd.collective_compute(
                    "AllGather", OP.bypass, replica_groups=RG,
                    ins=[cc_hT_in[0][:]], outs=[hT_all[0][:]],
                )

            # ---------------- helpers --------------------------------------
            def emit_ln(l, which, dstT, lpool, psp):
                """LayerNorm hbuf in place; optionally emit transposed copy.

                which: 0 -> LN1 (g1/be1), 1 -> LN2 (g2/be2)
                dstT:  None or SBUF tile [P, 8, NS] (f32r) for transposed out
                """
                if has_gb:
                    g_d = (g1_d, g2_d)[which]
                    be_d = (be1_d, be2_d)[which]
                for i in range(2):
                    x = hbuf[i]
                    bst = lpool.tile([P, 2, 6], F32, tag="bst")
                    for ch in range(2):
                        nc.vector.bn_stats(
                            bst[:, ch, :], x[:, ch * 512:(ch + 1) * 512]
                        )
                    mv = lpool.tile([P, 2], F32, tag="mv")
                    nc.vector.bn_aggr(mv[:], bst[:])
                    lnv = lpool.tile([P, 1], F32, tag="lnv")
                    # ddof=1 correction folded into Ln's input scale
                    nc.scalar.activation(
                        lnv[:], mv[:, 1:2], AF.Ln, scale=DM / (DM - 1.0)
                    )
                    rstd = lpool.tile([P, 1], F32, tag="rstd")
                    nc.scalar.activation(rstd[:], lnv[:], AF.Exp, scale=-0.5)
                    if not has_gb:
                        nc.vector.tensor_scalar(
                            x[:], x[:], mv[:, 0:1], rstd[:],
                            OP.subtract, OP.mult,
                        )
                    else:
                        u = lpool.tile([P, DM], F32, tag="lnu")
                        nc.vector.tensor_scalar(
                            u[:], x[:], mv[:, 0:1], rstd[:],
                            OP.subtract, OP.mult,
                        )
                        gt = lpool.tile([P, DM], F32, tag="lngt")
                        nc.sync.dma_start(gt[:], g_d[l, i * P:(i + 1) * P, :])
                        bt = lpool.tile([P, DM], F32, tag="lnbt")
                        nc.sync.dma_start(bt[:], be_d[l, i * P:(i + 1) * P, :])
                        nc.vector.tensor_mul(u[:], u[:], gt[:])
                        nc.vector.tensor_add(x[:], u[:], bt[:])
                    if dstT is not None:
                        for dc in range(C):
                            tp = psp.tile([P, P], F32, tag="trps")
                            nc.tensor.transpose(
                                tp[:], x[:, dc * P:(dc + 1) * P], idt[:]
                            )
                            nc.scalar.activation(
                                dstT[:, dc, i * P:(i + 1) * P], tp[:], AF.Copy
                            )

            # ---------------- layers ----------------------------------------
            for l in range(L):
                with tc.tile_pool(name=f"lay{l}", bufs=1) as lp:
                    QT = lp.tile([P, N], F32R, tag="QT")
                    KT = lp.tile([P, N], F32R, tag="KT")
                    Vm = lp.tile([P, 16, P], F32R, tag="Vm")
                    h2T = lp.tile([P, C, NS], F32R, tag="h2T")

                    # ---- QKV projections ----
                    wqt = wq_g.tile([P, C, P], F32R, tag="wqt")
                    nc.sync.dma_start(
                        wqt[:], wq_d[l].rearrange("(dc p) f -> p dc f", p=P)
                    )
                    wkt = wq_g.tile([P, C, P], F32R, tag="wkt")
                    nc.sync.dma_start(
                        wkt[:], wk_d[l].rearrange("(dc p) f -> p dc f", p=P)
                    )
                    wvt = wq_g.tile([P, C, P], F32R, tag="wvt")
                    nc.sync.dma_start(
                        wvt[:], wv_d[l].rearrange("(dc p) f -> p dc f", p=P)
                    )
                    bqc = wq_g.tile([P, 1], F32, tag="bqc")
                    nc.sync.dma_start(bqc[:], bq_d[l])
                    bkc = wq_g.tile([P, 1], F32, tag="bkc")
                    nc.sync.dma_start(bkc[:], bk_d[l])
                    bvc = wq_g.tile([P, 1], F32, tag="bvc")
                    nc.sync.dma_start(bvc[:], bv_d[l])

                    with (
                        tc.tile_pool(name="qkv", bufs=3) as qkvp,
                        tc.tile_pool(name="qkvps", bufs=3, space="PSUM") as qps,
                    ):
                        VTf = qkvp.tile([P, N], F32, tag="VTf", bufs=1)
                        projs = [
                            (wqt, bqc, QT, F32R),
                            (wkt, bkc, KT, F32R),
                            (wvt, bvc, VTf, F32),
                        ]
                        for jp in range(4):
                            pss = [qps.tile([P, 512], F32, tag="qkvps", name=f"qkvps{jp}_{_t}") for _t in range(3)]
                            for half in range(2):
                                j = jp * 2 + half
                                hTb = qkvp.tile([P, C, NS], F32R, tag="hTb")
                                nc.sync.dma_start(
                                    hTb[:],
                                    hT_all[l][j * DM:(j + 1) * DM, :]
                                    .rearrange("(dc p) n -> p dc n", p=P)
                                    .bitcast(F32R),
                                )
                                for t in range(3):
                                    wt_ = projs[t][0]
                                    for dc in range(C):
                                        nc.tensor.matmul(
                                            pss[t][:, half * NS:(half + 1) * NS],
                                            wt_[:, dc, :],
                                            hTb[:, dc, :],
                                            start=(dc == 0), stop=(dc == C - 1),
                                        )
                            for t in range(3):
                                _, b_, dst, _dt = projs[t]
                                nc.scalar.activation(
                                    dst[:, jp * 512:(jp + 1) * 512], pss[t][:],
                                    AF.Identity, bias=b_[:],
                                )
                        # V: [v, m] -> [m, v] transposes
                        for mc in range(16):
                            tp = qps.tile([P, P], F32, tag="trps")
                            nc.tensor.transpose(
                                tp[:], VTf[:, mc * P:(mc + 1) * P], idt[:]
                            )
                            nc.scalar.activation(Vm[:, mc, :], tp[:], AF.Copy)

                    # ---- attention ----
                    # Scores transposed S^T[m, n]; log_softmax over n applied
                    # lazily.  n is processed in two half-passes so that the
                    # per-head attnT accumulators (base-partition-0 PSUM
                    # tiles) plus the S workspace fit in the 8 PSUM banks.
                    sums = lp.tile([P, HC, 16, 2], F32, tag="sums")
                    ZTh = [
                        lp.tile([64, N], F32R, tag=f"ZTh{h}", name=f"ZTh{h}")
                        for h in range(HC)
                    ]
                    with (
                        tc.tile_pool(name="attnps", bufs=1, space="PSUM") as aps,
                    ):
                      with (
                        tc.tile_pool(name="sloop", bufs=3) as slp,
                        tc.tile_pool(name="sloopps", bufs=2, space="PSUM") as sps_p,
                      ):
                        for nh in range(2):
                            attn_ps = [
                                aps.tile([64, 1024], F32, tag=f"attnps{h}",
                                         name=f"attnps{h}")
                                for h in range(HC)
                            ]
                            for mc in range(16):
                                for h in range(HC):
                                    r0 = h * 64
                                    sp = sps_p.tile([P, 1024], F32, tag="sps")
                                    for nb in range(2):
                                        ncol = (nh * 2 + nb) * 512
                                        nc.tensor.matmul(
                                            sp[:, nb * 512:(nb + 1) * 512],
                                            KT[r0:r0 + 64, mc * P:(mc + 1) * P],
                                            QT[r0:r0 + 64, ncol:ncol + 512],
                                            start=True, stop=True,
                                        )
                                    esc = slp.tile([P, 1024], F32, tag="esc", bufs=2)
                                    nc.scalar.activation(
                                        esc[:], sp[:], AF.Exp,
                                        accum_out=sums[:, h, mc, nh:nh + 1],
                                    )
                                    ssb = slp.tile([P, 1024], F32R, tag="ssb")
                                    nc.vector.tensor_copy(ssb[:], sp[:])
                                    for nb in range(2):
                                        nc.tensor.matmul(
                                            attn_ps[h][:, nb * 512:(nb + 1) * 512],
                                            Vm[:, mc, r0:r0 + 64],
                                            ssb[:, nb * 512:(nb + 1) * 512],
                                            start=(mc == 0), stop=(mc == 15),
                                            skip_group_check=True,
                                        )
                            # drain uncorrected halves to SBUF
                            for h in range(HC):
                                nc.vector.tensor_copy(
                                    ZTh[h][:, nh * 1024:(nh + 1) * 1024],
                                    attn_ps[h][:],
                                )
                      # logsumexp and rank-1 correction
                      with (
                            tc.tile_pool(name="corr", bufs=1) as cp,
                            tc.tile_pool(name="corrps", bufs=1, space="PSUM") as cps_p,
                      ):
                            sumt = cp.tile([P, HC, 16], F32, tag="sumt")
                            nc.vector.tensor_tensor(
                                sumt[:], sums[:, :, :, 0], sums[:, :, :, 1], OP.add
                            )
                            csb = cp.tile([P, HC, 16], F32R, tag="csb")
                            nc.scalar.activation(csb[:], sumt[:], AF.Ln)
                            corr_pair = cp.tile([1, P], F32, tag="corrpair")
                            for h in range(HC):
                                r0 = h * 64
                                cps = cps_p.tile([1, 64], F32, tag="corrps")
                                for mc in range(16):
                                    nc.tensor.matmul(
                                        cps[:],
                                        csb[:, h, mc:mc + 1],
                                        Vm[:, mc, r0:r0 + 64],
                                        start=(mc == 0), stop=(mc == 15),
                                    )
                                nc.scalar.activation(
                                    corr_pair[:, r0:r0 + 64], cps[:], AF.Copy
                                )
                            for h in range(HC):
                                ctp = cps_p.tile([64, 1], F32, tag="ctps")
                                nc.tensor.transpose(
                                    ctp[:], corr_pair[:, h * 64:(h + 1) * 64],
                                    idt[:1, :1],
                                )
                                corr_h = cp.tile([64, 1], F32, tag="corrh")
                                nc.scalar.activation(corr_h[:], ctp[:], AF.Copy)
                                nc.vector.tensor_scalar(
                                    ZTh[h][:], ZTh[h][:], corr_h[:], None,
                                    OP.subtract,
                                )
                                nc.sync.dma_start(
                                    cc_at_in[l]
                                    .rearrange("(j hp) n -> hp j n", hp=P)
                                    [h * 64:(h + 1) * 64]
                                    .bitcast(F32R),
                                    ZTh[h][:].rearrange(
                                        "p (j n) -> p j n", n=NS
                                    ),
                                )
                    nc.gpsimd.collective_compute(
                        "AllToAll", OP.bypass, replica_groups=RG,
                        ins=[cc_at_in[l][:]], outs=[cc_at_out[l][:]],
                    )

                    # ---- WO + residual + LN1 ----
                    wot = wo_g.tile([P, C, DM], F32R, tag="wot")
                    nc.sync.dma_start(
                        wot[:], wo_d[l].rearrange("(v p) d -> p v d", p=P)
                    )
                    with (
                        tc.tile_pool(name="wo", bufs=2) as wop,
                        tc.tile_pool(name="wops", bufs=2, space="PSUM") as wops,
                    ):
                        zta = wop.tile([P, C, NS], F32R, tag="zta")
                        nc.sync.dma_start(
                            zta[:],
                            cc_at_out[l]
                            .rearrange("(j p) n -> p j n", p=P)
                            .bitcast(F32R),
                        )
                        if has_bo_b2:
                            bot = wop.tile([P, DM], F32, tag="bot")
                            nc.sync.dma_start(bot[:], bo_d[l])
                        for i in range(2):
                            for do in range(2):
                                ps = wops.tile([P, 512], F32, tag="wops")
                                for v in range(C):
                                    nc.tensor.matmul(
                                        ps[:],
                                        zta[:, v, i * P:(i + 1) * P],
                                        wot[:, v, do * 512:(do + 1) * 512],
                                        start=(v == 0), stop=(v == C - 1),
                                    )
                                dst = hbuf[i][:, do * 512:(do + 1) * 512]
                                nc.vector.tensor_tensor(dst, dst, ps[:], OP.add)
                                if has_bo_b2:
                                    nc.vector.tensor_tensor(
                                        dst, dst,
                                        bot[:, do * 512:(do + 1) * 512], OP.add,
                                    )
                        emit_ln(l, 0, h2T, wop, wops)

                    # ---- FFN ----
                    with (
                        tc.tile_pool(name="ffn", bufs=2) as fp,
                        tc.tile_pool(name="ffnps", bufs=2, space="PSUM") as fps,
                        tc.tile_pool(name="w2psp", bufs=1, space="PSUM") as w2psp,
                    ):
                        AT = fp.tile([P, DFF // P, NS], F32R, tag="AT", bufs=1)
                        for fc in range(DFF // P):
                            w1t = w12_g.tile([P, C, P], F32R, tag="w1t")
                            nc.sync.dma_start(
                                w1t[:],
                                w1_d[l][:, fc * P:(fc + 1) * P]
                                .rearrange("(dc p) f -> p dc f", p=P),
                            )
                            b1c = w12_g.tile([P, 1], F32, tag="b1c")
                            nc.sync.dma_start(b1c[:], b1_d[l, fc])
                            ps = fps.tile([P, NS], F32, tag="atps")
                            for dc in range(C):
                                nc.tensor.matmul(
                                    ps[:], w1t[:, dc, :], h2T[:, dc, :],
                                    start=(dc == 0), stop=(dc == C - 1),
                                )
                            nc.scalar.activation(
                                AT[:, fc, :], ps[:], AF.Relu, bias=b1c[:]
                            )
                        ps4 = [
                            w2psp.tile([P, 512], F32, tag=f"w2ps{k}", name=f"w2ps{k}")
                            for k in range(4)
                        ]
                        for fc in range(DFF // P):
                            w2t = w12_g.tile([P, DM], F32R, tag="w2t")
                            nc.sync.dma_start(
                                w2t[:], w2_d[l, fc * P:(fc + 1) * P, :]
                            )
                            for i in range(2):
                                for do in range(2):
                                    nc.tensor.matmul(
                                        ps4[i * 2 + do][:],
                                        AT[:, fc, i * P:(i + 1) * P],
                                        w2t[:, do * 512:(do + 1) * 512],
                                        start=(fc == 0), stop=(fc == DFF // P - 1),
                                        skip_group_check=True,
                                    )
                        if has_bo_b2:
                            b2t = fp.tile([P, DM], F32, tag="b2t")
                            nc.sync.dma_start(b2t[:], b2_d[l])
                        for i in range(2):
                            for do in range(2):
                                dst = hbuf[i][:, do * 512:(do + 1) * 512]
                                nc.vector.tensor_tensor(
                                    dst, dst, ps4[i * 2 + do][:], OP.add
                                )
                                if has_bo_b2:
                                    nc.vector.tensor_tensor(
                                        dst, dst,
                                        b2t[:, do * 512:(do + 1) * 512], OP.add,
                                    )
                        if l < L - 1:
                            hTsh2 = fp.tile([P, C, NS], F32R, tag="hTsh2", bufs=1)
                            emit_ln(l, 1, hTsh2, fp, fps)
                            nc.sync.dma_start(
                                cc_hT_in[l + 1]
                                .rearrange("(dc p) n -> p dc n", p=P)
                                .bitcast(F32R),
                                hTsh2[:],
                            )
                            nc.gpsimd.collective_compute(
                                "AllGather", OP.bypass, replica_groups=RG,
                                ins=[cc_hT_in[l + 1][:]], outs=[hT_all[l + 1][:]],
                            )
                        else:
                            emit_ln(l, 1, None, fp, fps)

            # ---------------- output ---------------------------------------
            for i in range(2):
                nc.sync.dma_start(out_d[i * P:(i + 1) * P, :], hbuf[i][:])

    nc.finalize()
    return nc


# ---------------------------------------------------------------------------
# host-side runner with persistent compiled executable
# ---------------------------------------------------------------------------

class _Runner:
    """Executes a finalized Bass program on n_cores via PJRT, reusing the
    compiled executable across calls (mirrors bass2jax.run_bass_via_pjrt)."""

    def __init__(self, nc, n_cores):
        import jax
        from jax.sharding import Mesh, PartitionSpec
        try:
            from jax.experimental.shard_map import shard_map
        except Exception:
            from jax.experimental import shard_map as _sm
            shard_map = _sm.shard_map

        bass2jax.install_neuronx_cc_hook()
        self.jax = jax
        self.nc = nc
        self.n_cores = n_cores

        partition_name = (
            nc.partition_id_tensor.name if nc.partition_id_tensor else None
        )
        in_names, out_names, out_avals, zero_outs = [], [], [], []
        for alloc in nc.m.functions[0].allocations:
            if not isinstance(alloc, mybir.MemoryLocationSet):
                continue
            name = alloc.memorylocations[0].name
            if alloc.kind == "ExternalInput":
                if name != partition_name:
                    in_names.append(name)
            elif alloc.kind == "ExternalOutput":
                shape = tuple(alloc.tensor_shape)
                dtype = mybir.dt.np(alloc.dtype)
                out_names.append(name)
                out_avals.append(jax.core.ShapedArray(shape, dtype))
                zero_outs.append(np.zeros(shape, dtype))
        self.in_names = list(in_names)
        self.out_names = out_names
        self.out_avals = out_avals
        self.zero_outs = zero_outs
        n_params = len(in_names)
        n_outs = len(out_avals)
        all_in_names = in_names + out_names
        if partition_name is not None:
            all_in_names = all_in_names + [partition_name]

        def _body(*args):
            operands = list(args)
            if partition_name is not None:
                operands.append(bass2jax.partition_id_tensor())
            outs = bass2jax._bass_exec_p.bind(
                *operands,
                out_avals=tuple(out_avals),
                in_names=tuple(all_in_names),
                out_names=tuple(out_names),
                lowering_input_output_aliases=(),
                sim_require_finite=True,
                sim_require_nnan=True,
                nc=nc,
            )
            return tuple(outs)

        devices = jax.devices()[:n_cores]
        assert len(devices) == n_cores
        self.mesh = Mesh(np.asarray(devices), ("core",))
        in_specs = (PartitionSpec("core"),) * (n_params + n_outs)
        out_specs = (PartitionSpec("core"),) * n_outs
        self.sharded = jax.jit(
            shard_map(
                _body, mesh=self.mesh, in_specs=in_specs, out_specs=out_specs,
                check_rep=False,
            ),
            donate_argnums=tuple(range(n_params, n_params + n_outs)),
            keep_unused=True,
        )

    def concat_inputs(self, in_maps):
        return [
            np.concatenate([np.asarray(m[name]) for m in in_maps], axis=0)
            for name in self.in_names
        ]

    def concat_zeros(self):
        return [
            np.zeros((self.n_cores * z.shape[0], *z.shape[1:]), z.dtype)
            for z in self.zero_outs
        ]

    def __call__(self, in_maps):
        out_arrs = self.sharded(*self.concat_inputs(in_maps), *self.concat_zeros())
        res = []
        for c in range(self.n_cores):
            res.append({
                name: np.asarray(out_arrs[i]).reshape(
                    self.n_cores, *self.out_avals[i].shape)[c]
                for i, name in enumerate(self.out_names)
            })
        return res


_CACHE = {}


def _get_runner(has_bo_b2, has_gb):
    key = (has_bo_b2, has_gb)
    if key not in _CACHE:
        nc = _build_program(has_bo_b2, has_gb)
        _CACHE[key] = _Runner(nc, C)
    return _CACHE[key]


# ---------------------------------------------------------------------------
# host-side input preparation
# ---------------------------------------------------------------------------

def _posenc():
    positions = (np.arange(N) + 1).astype(np.float32)
    factors = np.exp(
        np.arange(0, DM, 2).astype(np.float32) / DM * (-math.log(10000.0))
    ).astype(np.float32)
    terms = positions[:, None] * factors[None, :]
    pm = np.zeros((N, DM), np.float32)
    pm[:, 0::2] = np.sin(terms)
    pm[:, 1::2] = np.cos(terms)
    return pm


def make_in_maps(X, emb, WQ, bQ, WK, bK, WV, bV, WO, bO, W1, b1, W2, b2,
                 g1, be1, g2, be2):
    X = np.asarray(X)
    emb = np.asarray(emb, dtype=np.float32)
    h0_full = np.ascontiguousarray(emb[X.astype(np.int64)])  # [N, DM]
    pos_full = _posenc()

    WQ = np.asarray(WQ, np.float32)
    WK = np.asarray(WK, np.float32)
    WV = np.asarray(WV, np.float32)
    bQ = np.asarray(bQ, np.float32)
    bK = np.asarray(bK, np.float32)
    bV = np.asarray(bV, np.float32)
    WO = np.ascontiguousarray(np.asarray(WO, np.float32))
    bO = np.asarray(bO, np.float32)
    W1 = np.ascontiguousarray(np.asarray(W1, np.float32))
    b1 = np.asarray(b1, np.float32)
    W2 = np.ascontiguousarray(np.asarray(W2, np.float32))
    b2 = np.asarray(b2, np.float32)
    g1 = np.asarray(g1, np.float32)
    be1 = np.asarray(be1, np.float32)
    g2 = np.asarray(g2, np.float32)
    be2 = np.asarray(be2, np.float32)

    scale = 1.0 / math.sqrt(DK)
    has_bo_b2 = bool(np.any(bO) or np.any(b2))
    has_gb = bool(
        np.any(g1 != 1.0) or np.any(be1) or np.any(g2 != 1.0) or np.any(be2)
    )

    b1r = np.ascontiguousarray(b1.reshape(L, DFF // P, P, 1))

    in_maps = []
    for c in range(C):
        hsl = slice(2 * c, 2 * c + 2)
        # [L, 2, DM, DK] -> [L, DM, 2*DK]
        wq_c = np.ascontiguousarray(
            WQ[:, hsl].transpose(0, 2, 1, 3).reshape(L, DM, 2 * DK) * scale
        )
        wk_c = np.ascontiguousarray(
            WK[:, hsl].transpose(0, 2, 1, 3).reshape(L, DM, 2 * DK)
        )
        wv_c = np.ascontiguousarray(
            WV[:, hsl].transpose(0, 2, 1, 3).reshape(L, DM, 2 * DV)
        )
        bq_c = np.ascontiguousarray(
            (bQ[:, hsl].reshape(L, 2 * DK) * scale)[..., None]
        )
        bk_c = np.ascontiguousarray(bK[:, hsl].reshape(L, 2 * DK)[..., None])
        bv_c = np.ascontiguousarray(bV[:, hsl].reshape(L, 2 * DV)[..., None])
        m = {
            "h0": np.ascontiguousarray(h0_full[c * NS:(c + 1) * NS]),
            "pos": np.ascontiguousarray(pos_full[c * NS:(c + 1) * NS]),
            "wq": wq_c, "wk": wk_c, "wv": wv_c,
            "bq": bq_c, "bk": bk_c, "bv": bv_c,
            "wo": WO, "w1": W1, "b1": b1r, "w2": W2,
        }
        if has_bo_b2:
            m["bo_b"] = np.ascontiguousarray(
                np.broadcast_to(bO[:, None, :], (L, P, DM))
            )
            m["b2_b"] = np.ascontiguousarray(
                np.broadcast_to(b2[:, None, :], (L, P, DM))
            )
        if has_gb:
            m["g1s"] = np.ascontiguousarray(g1[:, c * NS:(c + 1) * NS])
            m["be1s"] = np.ascontiguousarray(be1[:, c * NS:(c + 1) * NS])
            m["g2s"] = np.ascontiguousarray(g2[:, c * NS:(c + 1) * NS])
            m["be2s"] = np.ascontiguousarray(be2[:, c * NS:(c + 1) * NS])
        in_maps.append(m)
    return in_maps, has_bo_b2, has_gb


def kernel(**inputs) -> np.ndarray:
    in_maps, has_bo_b2, has_gb = make_in_maps(**inputs)
    runner = _get_runner(has_bo_b2, has_gb)
    results = runner(in_maps)
    return np.concatenate([r["out"] for r in results], axis=0)


if __name__ == "__main__":
    # quick self-run with random-ish inputs
    rng = np.random.default_rng(0)
    inputs = {
        "X": rng.integers(0, VOCAB, size=(N,)),
        "emb": rng.standard_normal((VOCAB, DM), dtype=np.float32) * 0.02,
        "WQ": rng.standard_normal((L, H, DM, DK), dtype=np.float32) * 0.02,
        "bQ": np.zeros((L, H, DK), np.float32),
        "WK": rng.standard_normal((L, H, DM, DK), dtype=np.float32) * 0.02,
        "bK": np.zeros((L, H, DK), np.float32),
        "WV": rng.standard_normal((L, H, DM, DV), dtype=np.float32) * 0.02,
        "bV": np.zeros((L, H, DV), np.float32),
        "WO": rng.standard_normal((L, H * DV, DM), dtype=np.float32) * 0.02,
        "bO": np.zeros((L, DM), np.float32),
        "W1": rng.standard_normal((L, DM, DFF), dtype=np.float32) * 0.02,
        "b1": np.zeros((L, DFF), np.float32),
        "W2": rng.standard_normal((L, DFF, DM), dtype=np.float32) * 0.02,
        "b2": np.zeros((L, DM), np.float32),
        "g1": np.ones((L, N, DM), np.float32),
        "be1": np.zeros((L, N, DM), np.float32),
        "g2": np.ones((L, N, DM), np.float32),
        "be2": np.zeros((L, N, DM), np.float32),
    }
    out = kernel(**inputs)
    print("out", out.shape, out.dtype, np.abs(out).max())
